# revision 36
# baseline (speedup 1.0000x reference)
"""Causal self-attention (B=4, T=2048, C=1024, 16 heads, fp32) on 8 TRN2 NeuronCores.

Sharding: 8 cores = 4 batches x 2 head-groups (8 heads each).  Each core runs an
identical program on its (batch, head-group) shard.  All matmul operands are
bf16 (PSUM accumulation fp32): same TensorE rate as fp32r but FWL-eligible
weight loads, 2x DVE on 16-bit elementwise, and half the SBUF/DMA footprint
(inputs are pre-cast to bf16 on the host, so x/weights DMA straight into
place with no staging/cast pass).

  phase 1: QKV projection in 4 chunks of 512 columns of x^T (chunk == q-chunk).
           Q^T / K^T land in [head*64+d, T] layout, V in natural [T, hd] layout
           augmented with a persistent ones column (written once at init) so
           the attention AV matmul also produces the softmax denominator.
  phase 2: flash-style causal attention per head pair.  S^T[k,q] blocks via
           K=64 matmuls packed two heads per PE pass at row tile positions
           0/64 (concurrent sub-array execution), one strided-AP exp on
           ScalarE reading only the un-masked q-range, block-causal masking
           via a triangular bf16 mask multiply, diagonal S/AV matmuls
           narrowed to q >= 128*j (no fp32r >=256 constraint in bf16).
           O^T and the softmax denominators accumulate together in PSUM.
           O is copied out of PSUM immediately (bf16) and normalized
           out-of-line: reciprocal_approx_fast on the denominator row,
           gpsimd partition-broadcast, one tensor_mul into y^T (aliasing the
           dead Q^T chunk).
  phase 3: output projection from y^T, partial [T, C] per core, DMA'd out.

  Emission is piece-interleaved: attention steps for q-chunk qc are
  round-robined with phase-1 pieces of chunk qc+1 and projection pieces of
  chunk qc-1, with S emitted one k-block ahead of AV so the TensorE queue
  never head-of-line blocks on an exp.

Host side: per-batch pairs of partial outputs are summed (the 2-way
"all-reduce" of the row-sharded Wproj), plus the rank-1 bias correction
(bqkv_v @ Wproj + bproj) which commutes with attention because softmax rows
sum to one.  Softmax max-subtraction is skipped: scores are ~N(0,1) after
the 1/8 scale, so exp never overflows.
"""
import numpy as np
import ml_dtypes

import concourse.bass as bass  # noqa: F401  (bass must be imported before tile)
import concourse.tile as tile
from concourse import mybir
from concourse.bacc import Bacc
from concourse.bass_utils import run_bass_kernel_spmd

F32 = mybir.dt.float32
BF16 = mybir.dt.bfloat16
NPBF16 = ml_dtypes.bfloat16
EXP = mybir.ActivationFunctionType.Exp
LOG = mybir.ActivationFunctionType.Ln

B, T, C = 4, 2048, 1024
NH = 16          # total heads
D = 64           # head dim
G = 2            # head groups (cores per batch)
HPG = NH // G    # heads per group = 8
GC = HPG * D     # columns per group = 512
CT = C // 128    # contraction tiles = 8
QCW = 512        # q-chunk width == phase-1 chunk width
NQC = T // QCW   # 4 q-chunks
NTT = T // 128   # 16 t-tiles
NHP = HPG // 2   # head pairs per core = 4


def build(loop_k=None, probe=None):
    """loop_k: if set, wrap the whole kernel body in a hardware For_i loop
    (timing-only variant: amortizes host dispatch jitter over loop_k
    back-to-back executions on device).  probe: timing-only ablations
    ('norecip' replaces the softmax-denominator reciprocal with a copy)."""
    nc = Bacc()
    xT = nc.dram_tensor("xT", [C, T], BF16, kind="ExternalInput")
    wqk = nc.dram_tensor("wqk", [C, 2 * GC], BF16, kind="ExternalInput")
    wv = nc.dram_tensor("wv", [C, GC], BF16, kind="ExternalInput")
    wp = nc.dram_tensor("wp", [GC, C], BF16, kind="ExternalInput")
    bqk = nc.dram_tensor("bqk", [128, 2 * GC // 128], F32, kind="ExternalInput")
    # bf16 output halves the 8MB/core output DMA (host upcasts + sums f32)
    out = nc.dram_tensor("out", [T, C], BF16, kind="ExternalOutput")

    with tile.TileContext(nc) as tc:
        with (
            tc.tile_pool(name="persist", bufs=1) as pp,
            tc.tile_pool(name="xc", bufs=2) as xcp,
            tc.tile_pool(name="pt", bufs=6) as ptp,
            tc.tile_pool(name="osb", bufs=3) as osbp,
            tc.tile_pool(name="rb", bufs=3) as rbp,
            tc.tile_pool(name="ost", bufs=2) as ost,
            tc.tile_pool(name="ps", bufs=2, space="PSUM") as ps,
            tc.tile_pool(name="psS", bufs=2, space="PSUM") as psS,
            tc.tile_pool(name="psO", bufs=1, space="PSUM") as psO,
        ):
            # long-lived SBUF tensors.  QT[j][qc] doubles as y^T storage.
            QT = [[pp.tile([128, QCW], BF16, tag=f"qt{j}_{q}", name=f"qt{j}_{q}")
                   for q in range(NQC)] for j in range(NHP)]
            KT = [[pp.tile([128, QCW], BF16, tag=f"kt{j}_{q}", name=f"kt{j}_{q}")
                   for q in range(NQC)] for j in range(NHP)]
            YT = QT
            VA = [pp.tile([128, HPG, D + 1], BF16, tag=f"va{t}", name=f"va{t}")
                  for t in range(NTT)]
            WQK = [pp.tile([128, 2 * GC], BF16, tag=f"wqk{c}", name=f"wqk{c}")
                   for c in range(CT)]
            WV = [pp.tile([128, GC], BF16, tag=f"wv{c}", name=f"wv{c}")
                  for c in range(CT)]
            WP = [pp.tile([128, C], BF16, tag=f"wpr{j}", name=f"wpr{j}")
                  for j in range(GC // 128)]
            bqk_sb = pp.tile([128, 2 * GC // 128], F32)
            tri32 = pp.tile([128, 128], F32)
            tri2 = pp.tile([128, 2, 128], BF16)

            def emit_init():
                # input DMAs (weights + bias); x chunks DMA per phase-1 chunk
                nc.sync.dma_start(out=bqk_sb, in_=bqk[:])
                for c in range(CT):
                    nc.sync.dma_start(out=WQK[c],
                                      in_=wqk[128 * c:128 * (c + 1), :])
                    nc.sync.dma_start(out=WV[c],
                                      in_=wv[128 * c:128 * (c + 1), :])
                # ones column of VA: written once, never touched again
                for t in range(NTT):
                    nc.vector.memset(VA[t][:, :, D:D + 1], 1.0)
                # upper-triangular (keep k<=q) mask, replicated for both heads
                nc.vector.memset(tri32, 1.0)
                nc.gpsimd.affine_select(
                    out=tri32, in_=tri32, pattern=[[1, 128]],
                    compare_op=mybir.AluOpType.is_ge, fill=0.0,
                    base=0, channel_multiplier=-1,
                )
                for ph in range(2):
                    nc.vector.tensor_copy(tri2[:, ph, :], tri32)

            def load_wp():
                for j in range(GC // 128):
                    nc.sync.dma_start(out=WP[j],
                                      in_=wp[128 * j:128 * (j + 1), :])

            XC_by_ch = {}

            def phase1_struct(ch):
                """Structured emission pieces for QKV projection of x columns
                [512*ch, 512*(ch+1)): (dma, m_halves[8], v_halves[4])."""
                XC = [xcp.tile([128, QCW], BF16, tag=f"xc{c}", name=f"xc{c}")
                      for c in range(CT)]
                XC_by_ch[ch] = XC

                def dma_x():
                    for c in range(CT):
                        nc.sync.dma_start(
                            out=XC[c],
                            in_=xT[128 * c:128 * (c + 1),
                                   QCW * ch:QCW * (ch + 1)],
                        )

                accs = {}
                m_halves = []
                for m in range(2 * GC // 128):
                    def qk_half(m=m, lo=True):
                        def f():
                            if lo:
                                accs[m] = ps.tile([128, 512], F32, tag="pp",
                                                  name="pp")
                            acc = accs[m]
                            cs = range(0, CT // 2) if lo else range(CT // 2, CT)
                            for c in cs:
                                nc.tensor.matmul(
                                    acc, WQK[c][:, 128 * m:128 * (m + 1)],
                                    XC[c],
                                    start=(c == 0), stop=(c == CT - 1),
                                )
                            if not lo:
                                dst = (QT[m][ch] if m < NHP
                                       else KT[m - NHP][ch])
                                nc.vector.tensor_scalar_add(
                                    dst, acc, bqk_sb[:, m:m + 1]
                                )
                        return f
                    m_halves.append([qk_half(m, True), qk_half(m, False)])

                vaccs = {}
                v_halves = []
                for ti in range(QCW // 128):
                    def v_half(ti=ti, lo=True):
                        def f():
                            t = (QCW // 128) * ch + ti
                            if lo:
                                vaccs[ti] = ps.tile([128, 512], F32, tag="pp",
                                                    name="pp")
                            acc = vaccs[ti]
                            cs = range(0, CT // 2) if lo else range(CT // 2, CT)
                            for c in cs:
                                nc.tensor.matmul(
                                    acc, XC[c][:, 128 * ti:128 * (ti + 1)],
                                    WV[c],
                                    start=(c == 0), stop=(c == CT - 1),
                                )
                            if not lo:
                                nc.vector.tensor_copy(
                                    VA[t][:, :, 0:D],
                                    acc.rearrange("p (h d) -> p h d", h=HPG),
                                )
                        return f
                    v_halves.append([v_half(ti, True), v_half(ti, False)])
                return dma_x, m_halves, v_halves

            def phase1_flat(ch):
                dma_x, m_halves, v_halves = phase1_struct(ch)
                return [dma_x] + [f for p in m_halves + v_halves for f in p]

            def attention_steps(qc):
                """Per-head-pair emission step lists for attention of q-chunk
                qc, S one k-block ahead of AV."""
                by_hp = []
                for hp in range(NHP):
                    steps = []
                    kbmax = 4 * (qc + 1)
                    st = {}

                    def emit_norm(hp=hp, qc=qc, st=st):
                        def f():
                            O = st.pop("O")
                            O_sb = osbp.tile([D + 1, 2, QCW], F32, tag="osb",
                                             name="osb")
                            nc.vector.tensor_copy(O_sb, O)
                            rc = rbp.tile([1, 2, QCW], F32, tag="rc",
                                          name="rc")
                            # 1/denominator: DVE's iterative-divide
                            # reciprocal (~3.3us) in the ScalarE(exp)-bound
                            # late windows, ScalarE exp(-ln(d)) (~2.3us,
                            # same natural_log_exp_and_others table set as
                            # the softmax exp) in the TensorE-bound early
                            # windows — each window's slack engine pays.
                            if qc >= 2 and probe != "lnexp_all":
                                nc.vector.reciprocal(rc, O_sb[D:D + 1, :, :])
                            else:
                                ln_d = rbp.tile([1, 2, QCW], F32, tag="lnd",
                                                name="lnd")
                                nc.scalar.activation(
                                    out=ln_d, in_=O_sb[D:D + 1, :, :],
                                    func=LOG)
                                nc.scalar.activation(
                                    out=rc, in_=ln_d, func=EXP, scale=-1.0)
                            for ph in range(2):
                                rb = rbp.tile([64, QCW], F32, tag="rb",
                                              name="rb")
                                nc.gpsimd.partition_broadcast(
                                    rb, rc[0:1, ph, :])
                                nc.vector.tensor_mul(
                                    YT[hp][qc][64 * ph:64 * (ph + 1), :],
                                    O_sb[0:D, ph, :], rb,
                                )
                        return f

                    def s_step(kb, hp=hp, st=st):
                        j = kb - 4 * qc
                        s_off = 128 * j if j > 0 else 0

                        def f():
                            S = psS.tile([128, 2, QCW], F32, tag="s", name="s")
                            for ph in range(2):
                                p_sl = slice(64 * ph, 64 * (ph + 1))
                                nc.tensor.matmul(
                                    S[:, ph, s_off:],
                                    KT[hp][kb // 4][p_sl,
                                                    128 * (kb % 4):
                                                    128 * (kb % 4 + 1)],
                                    QT[hp][qc][p_sl, s_off:],
                                    start=True, stop=True,
                                )
                            P = ptp.tile([128, 2, QCW], BF16, tag="p",
                                         name="p")
                            nc.scalar.activation(
                                out=P[:, :, s_off:], in_=S[:, :, s_off:],
                                func=EXP, scale=0.125,
                            )
                            if j >= 0:
                                nc.vector.tensor_mul(
                                    P[:, :, s_off:s_off + 128],
                                    P[:, :, s_off:s_off + 128], tri2,
                                )
                            st[kb] = (P, s_off)
                        return f

                    def av_step(kb, hp=hp, st=st, kbmax=kbmax):
                        def f():
                            P, s_off = st.pop(kb)
                            if kb == 0:
                                st["O"] = psO.tile([D + 1, 2, QCW], F32,
                                                   tag="o", name="o")
                            O = st["O"]
                            for ph in range(2):
                                nc.tensor.matmul(
                                    O[:, ph, s_off:],
                                    VA[kb][:, 2 * hp + ph, :],
                                    P[:, ph, s_off:],
                                    start=(kb == 0), stop=(kb == kbmax - 1),
                                )
                        return f

                    steps.append(s_step(0))
                    for kb in range(kbmax):
                        fs = []
                        if kb + 1 < kbmax:
                            fs.append(s_step(kb + 1))
                        fs.append(av_step(kb))

                        def both(fs=fs):
                            def f():
                                for g in fs:
                                    g()
                            return f
                        steps.append(both())
                    steps.append(emit_norm())
                    by_hp.append(steps)
                return by_hp

            def proj_pieces(qc):
                pieces = []
                for ti in range(4):
                    for nn in range(2):
                        def f(ti=ti, nn=nn):
                            t = 4 * qc + ti
                            acc = ps.tile([128, 512], F32, tag="pp", name="pp")
                            for j in range(GC // 128):
                                nc.tensor.matmul(
                                    acc,
                                    YT[j][qc][:, 128 * ti:128 * (ti + 1)],
                                    WP[j][:, 512 * nn:512 * (nn + 1)],
                                    start=(j == 0), stop=(j == GC // 128 - 1),
                                )
                            o = ost.tile([128, 512], BF16, tag="o", name="o")
                            nc.vector.tensor_copy(o, acc)
                            nc.sync.dma_start(
                                out=out[128 * t:128 * (t + 1),
                                        512 * nn:512 * (nn + 1)],
                                in_=o,
                            )
                        pieces.append(f)
                return pieces

            def interleave(steps, fill):
                fi = 0
                for i, s in enumerate(steps):
                    s()
                    target = (i + 1) * len(fill) // len(steps)
                    while fi < target:
                        fill[fi]()
                        fi += 1
                while fi < len(fill):
                    fill[fi]()
                    fi += 1

            def emit_all():
                emit_init()
                # window 0: weave attention(0) into phase-1(0) so ScalarE
                # starts exp'ing ~3 pieces in instead of after the whole
                # chunk.  Emission prefix satisfies hp0's deps (m0, m4,
                # v0..v3); remaining m-pairs go at the FRONT of the fill so
                # hp1..3's deps land before their steps.
                dma0, m0, v0 = phase1_struct(0)
                att0 = attention_steps(0)
                dma0()
                for f in m0[0] + m0[4]:
                    f()
                for pair in v0:
                    for f in pair:
                        f()
                fill = [f for h in (1, 5, 2, 6, 3, 7) for f in m0[h]]
                fill += phase1_flat(1)
                fill.append(load_wp)
                interleave([s for hp in att0 for s in hp], fill)
                # windows 1..2: attention(qc) + phase-1(qc+1)
                for qc in (1, 2):
                    interleave(
                        [s for hp in attention_steps(qc) for s in hp],
                        phase1_flat(qc + 1),
                    )
                # window 3 is ScalarE(exp)-bound: park all deferred
                # projection work here to fill TensorE idle
                fill = proj_pieces(0) + proj_pieces(1) + proj_pieces(2)
                interleave(
                    [s for hp in attention_steps(3) for s in hp], fill)
                for p in proj_pieces(NQC - 1):
                    p()

            if loop_k:
                with tc.For_i(0, loop_k, 1):
                    emit_all()
            else:
                emit_all()
    nc.finalize()
    return nc


_NC = None


def _get_nc():
    global _NC
    if _NC is None:
        _NC = build()
    return _NC


def _bf(a):
    return np.ascontiguousarray(a.astype(NPBF16))


def _shard(x, Wqkv, bqkv, Wproj):
    in_maps = []
    for core in range(8):
        b, g = core // G, core % G
        cs = slice(GC * g, GC * (g + 1))
        wqk_h = np.concatenate([Wqkv[:, cs], Wqkv[:, C:][:, cs]], axis=1)
        bqk_h = np.concatenate([bqkv[cs], bqkv[C:][cs.start:cs.stop]])
        in_maps.append({
            "xT": _bf(x[b].T),
            "wqk": _bf(wqk_h),
            "wv": _bf(Wqkv[:, 2 * C:][:, cs]),
            "wp": _bf(Wproj[cs, :]),
            "bqk": np.ascontiguousarray(
                bqk_h.reshape(2 * GC // 128, 128).T.astype(np.float32)),
        })
    return in_maps


def kernel(x, Wqkv, bqkv, Wproj, bproj, _want_results=False, **run_kwargs):
    x = np.asarray(x, dtype=np.float32)
    Wqkv = np.asarray(Wqkv, dtype=np.float32)
    bqkv = np.asarray(bqkv, dtype=np.float32)
    Wproj = np.asarray(Wproj, dtype=np.float32)
    bproj = np.asarray(bproj, dtype=np.float32)

    nc = _get_nc()
    in_maps = _shard(x, Wqkv, bqkv, Wproj)
    res = run_bass_kernel_spmd(nc, in_maps, core_ids=list(range(8)),
                               **run_kwargs)

    out = np.empty((B, T, C), dtype=np.float32)
    for b in range(B):
        out[b] = np.asarray(res.results[G * b]["out"], dtype=np.float32)
        for g in range(1, G):
            out[b] += np.asarray(res.results[G * b + g]["out"],
                                 dtype=np.float32)
    # rank-1 corrections: v-bias (rows of softmax sum to 1) and proj bias
    out += bqkv[2 * C:] @ Wproj + bproj
    if _want_results:
        return out, res
    return out


# revision 38
# speedup vs baseline: 1.0216x; 1.0216x over previous
"""Causal self-attention (B=4, T=2048, C=1024, 16 heads, fp32) on 8 TRN2 NeuronCores.

Sharding: 8 cores = 4 batches x 2 head-groups (8 heads each).  Each core runs an
identical program on its (batch, head-group) shard.  All matmul operands are
bf16 (PSUM accumulation fp32): same TensorE rate as fp32r but FWL-eligible
weight loads, 2x DVE on 16-bit elementwise, and half the SBUF/DMA footprint
(inputs are pre-cast to bf16 on the host, so x/weights DMA straight into
place with no staging/cast pass).

  phase 1: QKV projection in 4 chunks of 512 columns of x^T (chunk == q-chunk).
           Q^T / K^T land in [head*64+d, T] layout, V in natural [T, hd] layout
           augmented with a persistent ones column (written once at init) so
           the attention AV matmul also produces the softmax denominator.
  phase 2: flash-style causal attention per head pair.  S^T[k,q] blocks via
           K=64 matmuls packed two heads per PE pass at row tile positions
           0/64 (concurrent sub-array execution), one strided-AP exp on
           ScalarE reading only the un-masked q-range, block-causal masking
           via a triangular bf16 mask multiply, diagonal S/AV matmuls
           narrowed to q >= 128*j (no fp32r >=256 constraint in bf16).
           O^T and the softmax denominators accumulate together in PSUM.
           O is copied out of PSUM immediately (bf16) and normalized
           out-of-line: reciprocal_approx_fast on the denominator row,
           gpsimd partition-broadcast, one tensor_mul into y^T (aliasing the
           dead Q^T chunk).
  phase 3: output projection from y^T, partial [T, C] per core, DMA'd out.

  Emission is piece-interleaved: attention steps for q-chunk qc are
  round-robined with phase-1 pieces of chunk qc+1 and projection pieces of
  chunk qc-1, with S emitted one k-block ahead of AV so the TensorE queue
  never head-of-line blocks on an exp.

Host side: per-batch pairs of partial outputs are summed (the 2-way
"all-reduce" of the row-sharded Wproj), plus the rank-1 bias correction
(bqkv_v @ Wproj + bproj) which commutes with attention because softmax rows
sum to one.  Softmax max-subtraction is skipped: scores are ~N(0,1) after
the 1/8 scale, so exp never overflows.
"""
import numpy as np
import ml_dtypes

import concourse.bass as bass  # noqa: F401  (bass must be imported before tile)
import concourse.tile as tile
from concourse import mybir
from concourse.bacc import Bacc
from concourse.bass_utils import run_bass_kernel_spmd

F32 = mybir.dt.float32
BF16 = mybir.dt.bfloat16
NPBF16 = ml_dtypes.bfloat16
EXP = mybir.ActivationFunctionType.Exp
LOG = mybir.ActivationFunctionType.Ln

B, T, C = 4, 2048, 1024
NH = 16          # total heads
D = 64           # head dim
G = 2            # head groups (cores per batch)
HPG = NH // G    # heads per group = 8
GC = HPG * D     # columns per group = 512
CT = C // 128    # contraction tiles = 8
QCW = 512        # q-chunk width == phase-1 chunk width
NQC = T // QCW   # 4 q-chunks
NTT = T // 128   # 16 t-tiles
NHP = HPG // 2   # head pairs per core = 4


def build(loop_k=None, probe=None):
    """loop_k: if set, wrap the whole kernel body in a hardware For_i loop
    (timing-only variant: amortizes host dispatch jitter over loop_k
    back-to-back executions on device).  probe: timing-only ablations
    ('norecip' replaces the softmax-denominator reciprocal with a copy)."""
    nc = Bacc()
    xT = nc.dram_tensor("xT", [C, T], BF16, kind="ExternalInput")
    wqk = nc.dram_tensor("wqk", [C, 2 * GC], BF16, kind="ExternalInput")
    wv = nc.dram_tensor("wv", [C, GC], BF16, kind="ExternalInput")
    wp = nc.dram_tensor("wp", [GC, C], BF16, kind="ExternalInput")
    bqk = nc.dram_tensor("bqk", [128, 2 * GC // 128], F32, kind="ExternalInput")
    out = nc.dram_tensor("out", [T, C], F32, kind="ExternalOutput")

    with tile.TileContext(nc) as tc:
        with (
            tc.tile_pool(name="persist", bufs=1) as pp,
            tc.tile_pool(name="xc", bufs=3) as xcp,
            tc.tile_pool(name="pt", bufs=8) as ptp,
            tc.tile_pool(name="osb", bufs=3) as osbp,
            tc.tile_pool(name="rb", bufs=3) as rbp,
            tc.tile_pool(name="ost", bufs=3) as ost,
            tc.tile_pool(name="ps", bufs=2, space="PSUM") as ps,
            tc.tile_pool(name="psS", bufs=2, space="PSUM") as psS,
            tc.tile_pool(name="psO", bufs=1, space="PSUM") as psO,
        ):
            # long-lived SBUF tensors.  QT[j][qc] doubles as y^T storage.
            QT = [[pp.tile([128, QCW], BF16, tag=f"qt{j}_{q}", name=f"qt{j}_{q}")
                   for q in range(NQC)] for j in range(NHP)]
            KT = [[pp.tile([128, QCW], BF16, tag=f"kt{j}_{q}", name=f"kt{j}_{q}")
                   for q in range(NQC)] for j in range(NHP)]
            YT = QT
            VA = [pp.tile([128, HPG, D + 1], BF16, tag=f"va{t}", name=f"va{t}")
                  for t in range(NTT)]
            WQK = [pp.tile([128, 2 * GC], BF16, tag=f"wqk{c}", name=f"wqk{c}")
                   for c in range(CT)]
            WV = [pp.tile([128, GC], BF16, tag=f"wv{c}", name=f"wv{c}")
                  for c in range(CT)]
            WP = [pp.tile([128, C], BF16, tag=f"wpr{j}", name=f"wpr{j}")
                  for j in range(GC // 128)]
            bqk_sb = pp.tile([128, 2 * GC // 128], F32)
            tri32 = pp.tile([128, 128], F32)
            tri2 = pp.tile([128, 2, 128], BF16)

            def emit_init():
                # input DMAs (weights + bias); x chunks DMA per phase-1 chunk
                nc.sync.dma_start(out=bqk_sb, in_=bqk[:])
                for c in range(CT):
                    nc.sync.dma_start(out=WQK[c],
                                      in_=wqk[128 * c:128 * (c + 1), :])
                    nc.sync.dma_start(out=WV[c],
                                      in_=wv[128 * c:128 * (c + 1), :])
                # ones column of VA: written once, never touched again
                for t in range(NTT):
                    nc.vector.memset(VA[t][:, :, D:D + 1], 1.0)
                # upper-triangular (keep k<=q) mask, replicated for both heads
                nc.vector.memset(tri32, 1.0)
                nc.gpsimd.affine_select(
                    out=tri32, in_=tri32, pattern=[[1, 128]],
                    compare_op=mybir.AluOpType.is_ge, fill=0.0,
                    base=0, channel_multiplier=-1,
                )
                for ph in range(2):
                    nc.vector.tensor_copy(tri2[:, ph, :], tri32)

            def load_wp():
                for j in range(GC // 128):
                    nc.sync.dma_start(out=WP[j],
                                      in_=wp[128 * j:128 * (j + 1), :])

            XC_by_ch = {}

            def phase1_struct(ch):
                """Structured emission pieces for QKV projection of x columns
                [512*ch, 512*(ch+1)): (dma, m_halves[8], v_halves[4])."""
                XC = [xcp.tile([128, QCW], BF16, tag=f"xc{c}", name=f"xc{c}")
                      for c in range(CT)]
                XC_by_ch[ch] = XC

                def dma_x():
                    for c in range(CT):
                        nc.sync.dma_start(
                            out=XC[c],
                            in_=xT[128 * c:128 * (c + 1),
                                   QCW * ch:QCW * (ch + 1)],
                        )

                accs = {}
                m_halves = []
                for m in range(2 * GC // 128):
                    def qk_half(m=m, lo=True):
                        def f():
                            if lo:
                                accs[m] = ps.tile([128, 512], F32, tag="pp",
                                                  name="pp")
                            acc = accs[m]
                            cs = range(0, CT // 2) if lo else range(CT // 2, CT)
                            for c in cs:
                                nc.tensor.matmul(
                                    acc, WQK[c][:, 128 * m:128 * (m + 1)],
                                    XC[c],
                                    start=(c == 0), stop=(c == CT - 1),
                                )
                            if not lo:
                                dst = (QT[m][ch] if m < NHP
                                       else KT[m - NHP][ch])
                                nc.vector.tensor_scalar_add(
                                    dst, acc, bqk_sb[:, m:m + 1]
                                )
                        return f
                    m_halves.append([qk_half(m, True), qk_half(m, False)])

                vaccs = {}
                v_halves = []
                for ti in range(QCW // 128):
                    def v_half(ti=ti, lo=True):
                        def f():
                            t = (QCW // 128) * ch + ti
                            if lo:
                                vaccs[ti] = ps.tile([128, 512], F32, tag="pp",
                                                    name="pp")
                            acc = vaccs[ti]
                            cs = range(0, CT // 2) if lo else range(CT // 2, CT)
                            for c in cs:
                                nc.tensor.matmul(
                                    acc, XC[c][:, 128 * ti:128 * (ti + 1)],
                                    WV[c],
                                    start=(c == 0), stop=(c == CT - 1),
                                )
                            if not lo:
                                nc.vector.tensor_copy(
                                    VA[t][:, :, 0:D],
                                    acc.rearrange("p (h d) -> p h d", h=HPG),
                                )
                        return f
                    v_halves.append([v_half(ti, True), v_half(ti, False)])
                return dma_x, m_halves, v_halves

            def phase1_flat(ch):
                dma_x, m_halves, v_halves = phase1_struct(ch)
                return [dma_x] + [f for p in m_halves + v_halves for f in p]

            def attention_steps(qc):
                """Per-head-pair emission step lists for attention of q-chunk
                qc, S one k-block ahead of AV."""
                by_hp = []
                for hp in range(NHP):
                    steps = []
                    kbmax = 4 * (qc + 1)
                    st = {}

                    def emit_norm(hp=hp, qc=qc, st=st):
                        def f():
                            O = st.pop("O")
                            O_sb = osbp.tile([D + 1, 2, QCW], F32, tag="osb",
                                             name="osb")
                            nc.vector.tensor_copy(O_sb, O)
                            rc = rbp.tile([1, 2, QCW], F32, tag="rc",
                                          name="rc")
                            # 1/denominator: DVE's iterative-divide
                            # reciprocal (~3.3us) in the ScalarE(exp)-bound
                            # late windows, ScalarE exp(-ln(d)) (~2.3us,
                            # same natural_log_exp_and_others table set as
                            # the softmax exp) in the TensorE-bound early
                            # windows — each window's slack engine pays.
                            if qc >= 2 and probe != "lnexp_all":
                                nc.vector.reciprocal(rc, O_sb[D:D + 1, :, :])
                            else:
                                ln_d = rbp.tile([1, 2, QCW], F32, tag="lnd",
                                                name="lnd")
                                nc.scalar.activation(
                                    out=ln_d, in_=O_sb[D:D + 1, :, :],
                                    func=LOG)
                                nc.scalar.activation(
                                    out=rc, in_=ln_d, func=EXP, scale=-1.0)
                            for ph in range(2):
                                rb = rbp.tile([64, QCW], F32, tag="rb",
                                              name="rb")
                                nc.gpsimd.partition_broadcast(
                                    rb, rc[0:1, ph, :])
                                nc.vector.tensor_mul(
                                    YT[hp][qc][64 * ph:64 * (ph + 1), :],
                                    O_sb[0:D, ph, :], rb,
                                )
                        return f

                    def s_step(kb, hp=hp, st=st):
                        j = kb - 4 * qc
                        s_off = 128 * j if j > 0 else 0

                        def f():
                            S = psS.tile([128, 2, QCW], F32, tag="s", name="s")
                            for ph in range(2):
                                p_sl = slice(64 * ph, 64 * (ph + 1))
                                nc.tensor.matmul(
                                    S[:, ph, s_off:],
                                    KT[hp][kb // 4][p_sl,
                                                    128 * (kb % 4):
                                                    128 * (kb % 4 + 1)],
                                    QT[hp][qc][p_sl, s_off:],
                                    start=True, stop=True,
                                )
                            P = ptp.tile([128, 2, QCW], BF16, tag="p",
                                         name="p")
                            nc.scalar.activation(
                                out=P[:, :, s_off:], in_=S[:, :, s_off:],
                                func=EXP, scale=0.125,
                            )
                            if j >= 0:
                                nc.vector.tensor_mul(
                                    P[:, :, s_off:s_off + 128],
                                    P[:, :, s_off:s_off + 128], tri2,
                                )
                            st[kb] = (P, s_off)
                        return f

                    def av_step(kb, hp=hp, st=st, kbmax=kbmax):
                        def f():
                            P, s_off = st.pop(kb)
                            if kb == 0:
                                st["O"] = psO.tile([D + 1, 2, QCW], F32,
                                                   tag="o", name="o")
                            O = st["O"]
                            for ph in range(2):
                                nc.tensor.matmul(
                                    O[:, ph, s_off:],
                                    VA[kb][:, 2 * hp + ph, :],
                                    P[:, ph, s_off:],
                                    start=(kb == 0), stop=(kb == kbmax - 1),
                                )
                        return f

                    steps.append(s_step(0))
                    for kb in range(kbmax):
                        fs = []
                        if kb + 1 < kbmax:
                            fs.append(s_step(kb + 1))
                        fs.append(av_step(kb))

                        def both(fs=fs):
                            def f():
                                for g in fs:
                                    g()
                            return f
                        steps.append(both())
                    steps.append(emit_norm())
                    by_hp.append(steps)
                return by_hp

            def proj_pieces(qc):
                pieces = []
                for ti in range(4):
                    for nn in range(2):
                        def f(ti=ti, nn=nn):
                            t = 4 * qc + ti
                            acc = ps.tile([128, 512], F32, tag="pp", name="pp")
                            for j in range(GC // 128):
                                nc.tensor.matmul(
                                    acc,
                                    YT[j][qc][:, 128 * ti:128 * (ti + 1)],
                                    WP[j][:, 512 * nn:512 * (nn + 1)],
                                    start=(j == 0), stop=(j == GC // 128 - 1),
                                )
                            o = ost.tile([128, 512], F32, tag="o", name="o")
                            nc.vector.tensor_copy(o, acc)
                            nc.sync.dma_start(
                                out=out[128 * t:128 * (t + 1),
                                        512 * nn:512 * (nn + 1)],
                                in_=o,
                            )
                        pieces.append(f)
                return pieces

            def interleave(steps, fill):
                fi = 0
                for i, s in enumerate(steps):
                    s()
                    target = (i + 1) * len(fill) // len(steps)
                    while fi < target:
                        fill[fi]()
                        fi += 1
                while fi < len(fill):
                    fill[fi]()
                    fi += 1

            def emit_all():
                emit_init()
                # window 0: weave attention(0) into phase-1(0) so ScalarE
                # starts exp'ing ~3 pieces in instead of after the whole
                # chunk.  Emission prefix satisfies hp0's deps (m0, m4,
                # v0..v3); remaining m-pairs go at the FRONT of the fill so
                # hp1..3's deps land before their steps.
                dma0, m0, v0 = phase1_struct(0)
                att0 = attention_steps(0)
                dma0()
                for f in m0[0] + m0[4]:
                    f()
                for pair in v0:
                    for f in pair:
                        f()
                fill = [f for h in (1, 5, 2, 6, 3, 7) for f in m0[h]]
                fill += phase1_flat(1)
                fill.append(load_wp)
                interleave([s for hp in att0 for s in hp], fill)
                # windows 1..2: attention(qc) + phase-1(qc+1)
                for qc in (1, 2):
                    interleave(
                        [s for hp in attention_steps(qc) for s in hp],
                        phase1_flat(qc + 1),
                    )
                # window 3 is ScalarE(exp)-bound: park all deferred
                # projection work here to fill TensorE idle
                fill = proj_pieces(0) + proj_pieces(1) + proj_pieces(2)
                interleave(
                    [s for hp in attention_steps(3) for s in hp], fill)
                for p in proj_pieces(NQC - 1):
                    p()

            if loop_k:
                with tc.For_i(0, loop_k, 1):
                    emit_all()
            else:
                emit_all()
    nc.finalize()
    return nc


_NC = None


def _get_nc():
    global _NC
    if _NC is None:
        _NC = build()
    return _NC


def _bf(a):
    return np.ascontiguousarray(a.astype(NPBF16))


def _shard(x, Wqkv, bqkv, Wproj):
    in_maps = []
    for core in range(8):
        b, g = core // G, core % G
        cs = slice(GC * g, GC * (g + 1))
        wqk_h = np.concatenate([Wqkv[:, cs], Wqkv[:, C:][:, cs]], axis=1)
        bqk_h = np.concatenate([bqkv[cs], bqkv[C:][cs.start:cs.stop]])
        in_maps.append({
            "xT": _bf(x[b].T),
            "wqk": _bf(wqk_h),
            "wv": _bf(Wqkv[:, 2 * C:][:, cs]),
            "wp": _bf(Wproj[cs, :]),
            "bqk": np.ascontiguousarray(
                bqk_h.reshape(2 * GC // 128, 128).T.astype(np.float32)),
        })
    return in_maps


def kernel(x, Wqkv, bqkv, Wproj, bproj, _want_results=False, **run_kwargs):
    x = np.asarray(x, dtype=np.float32)
    Wqkv = np.asarray(Wqkv, dtype=np.float32)
    bqkv = np.asarray(bqkv, dtype=np.float32)
    Wproj = np.asarray(Wproj, dtype=np.float32)
    bproj = np.asarray(bproj, dtype=np.float32)

    nc = _get_nc()
    in_maps = _shard(x, Wqkv, bqkv, Wproj)
    res = run_bass_kernel_spmd(nc, in_maps, core_ids=list(range(8)),
                               **run_kwargs)

    out = np.empty((B, T, C), dtype=np.float32)
    for b in range(B):
        out[b] = res.results[G * b]["out"]
        for g in range(1, G):
            out[b] += res.results[G * b + g]["out"]
    # rank-1 corrections: v-bias (rows of softmax sum to 1) and proj bias
    out += bqkv[2 * C:] @ Wproj + bproj
    if _want_results:
        return out, res
    return out
